# revision 14
# baseline (speedup 1.0000x reference)
"""Trainium2 Bass kernel for GNN message-passing attention MLP.

Computation (per node b with N=32 neighbors, F=128 features):
  h_nb   = relu(input1 @ W_nb + b_nb)          [B,N,H1]
  h_self = relu(input2 @ W_self + b_self)      [B,H1]
  z      = relu(h_nb @ W_a1[:H1] + h_self @ W_a1[H1:] + b_a1)   [B,N,H2]
  out    = (z @ W_a2 + b_a2).reshape(B*N, 1)

Strategy: data-parallel over 8 NeuronCores (6250 nodes each, padded to
6272).  Host-side prep quantizes input1 to fp8-e3m4 (x2 scale, clipped;
the 1/2 folded into W_nb) and pre-transposes to [F, neighbor, node]
layout, halving the dominant HBM stream; the tiny self path
ys = h_self @ W_a1[H1:] + b_a1 is computed on host.  On device, nodes
ride the matmul free dim (512-wide superblocks, one 8-neighbor x tile
per burst alternating across the two HWDGE rings).  mm1 runs as
column-tiled fp8 pairs into double-bank PSUM tiles (one relu
instruction covers 4 neighbors); z tiles pack 8 neighbors (2 per
32-partition strip via block-diagonal stationaries, K=128) and are
evacuated with the shifted relu max(z_nb, -ys) on the DVE — using
relu(z_nb + ys) = max(z_nb, -ys) + ys, whose linear +ys term commutes
with the final H2-contraction and is added per-node on the host
together with b_a2.  The final contraction runs as one batched group
of 4 concurrent column tiles; its [neighbor, node] strips are
re-transposed on the host.  The per-burst z-flush is emitted in two
halves interleaved into the next burst so the PE never stalls on the
relu engines.
"""

import sys
import types

import numpy as np
import ml_dtypes

import concourse.bass as bass
import concourse.mybir as mybir
from concourse import bacc
from concourse.tile import TileContext
from concourse.bass_utils import run_bass_kernel_spmd


def _ensure_axon_hooks():
    """bass_utils' trace path imports antenv.axon_hooks, which this image
    lacks; synthesize it (wired to the PJRT plugin's NTFF profiler) so a
    BASS_TRACE=1 environment doesn't crash the run."""
    try:
        import antenv.axon_hooks  # noqa: F401
        return
    except ImportError:
        pass
    try:
        import antenv
        mod = types.ModuleType("antenv.axon_hooks")
        holder = [None]
        mod.set_axon_ntff_profile_hook = lambda h: holder.__setitem__(0, h)
        mod.get_axon_ntff_profile_hook = lambda: holder[0]
        sys.modules["antenv.axon_hooks"] = mod
        antenv.axon_hooks = mod
        try:
            from trn_agent_boot.trn_boot import _ntff_profile_via_ctypes
            mod.set_axon_ntff_profile_hook(
                _ntff_profile_via_ctypes("/opt/axon/libaxon_pjrt.so"))
        except Exception:
            pass
    except Exception:
        pass


_ensure_axon_hooks()

BF16 = ml_dtypes.bfloat16
E3M4 = ml_dtypes.float8_e3m4
XSCALE = 2.0                   # input1 prescale before e3m4 cast
E3MAX = 15.5                   # e3m4 saturation
WA_PSUM_DMA = False            # PSUM-source DMA rejected by bass; use DVE copy

B, N, F = 50000, 32, 128
H1, H2 = 64, 16
N_CORES = 8
B_SH = B // N_CORES            # 6250 nodes per core
B_PAD = 6272                   # padded to 49*128
SB = 512                       # superblock: nodes per compute block
SBS = [(s * SB, SB) for s in range(B_PAD // SB)]
_rem = B_PAD - (B_PAD // SB) * SB
if _rem:
    SBS.append(((B_PAD // SB) * SB, _rem))
R_PAD = B_PAD * N              # padded rows per core (200704)
R_SH = B_SH * N                # valid rows per core (200000)

_cache = {}
last_results = None  # BassKernelResults of the most recent run (for test harness)
TRACE = False        # set True from test harness to capture an HW profile


def _build_graph():
    dt = mybir.dt
    nc = bacc.Bacc("TRN2", target_bir_lowering=False, debug=False,
                   num_devices=N_CORES)

    xt = nc.declare_dram_parameter("xt", [128 * N * B_PAD], dt.float8e3, isOutput=False)
    ysr = nc.declare_dram_parameter("ysr", [128 * B_PAD], dt.bfloat16, isOutput=False)
    wpack = nc.declare_dram_parameter("wpack", [128, 128], dt.bfloat16, isOutput=False)
    bnb = nc.declare_dram_parameter("bnb", [128, 1], dt.float32, isOutput=False)
    out = nc.declare_dram_parameter("out", [R_PAD], dt.float32, isOutput=True)

    RELU = mybir.ActivationFunctionType.Relu

    with TileContext(nc) as tc:
        with tc.tile_pool(name="const", bufs=1) as cpool, \
             tc.tile_pool(name="xp", bufs=3) as xpool, \
             tc.tile_pool(name="hp", bufs=8) as hpool, \
             tc.tile_pool(name="zs", bufs=10) as zpool, \
             tc.tile_pool(name="wp", bufs=2) as wpool, \
             tc.tile_pool(name="psum", bufs=1, space="PSUM") as ppool:
            # PSUM budget (8 banks): hp double x2 (4), zp x2, wa x2.

            wpack_sb = cpool.tile([128, 128], dt.bfloat16)
            nc.scalar.dma_start(out=wpack_sb[:], in_=wpack[:])
            wnb_sb = wpack_sb[:, 0:H1]
            w2a_sb = wpack_sb[:, 64:96]
            wg3_sb = wpack_sb[:, 96:128]
            bnb_sb = cpool.tile([128, 1], dt.float32)
            nc.scalar.dma_start(out=bnb_sb[:], in_=bnb[:])

            first = True
            relu_rr = [0]     # round-robin for h relu engine assignment
            xrr = [0]         # round-robin for x chunk DMA ring assignment
            pend = None       # deferred z-flush of the previous burst
            pend_out = None   # deferred mm3 + wa evac of the previous sb

            def emit_flush(f, part):
                # z-tile for one burst, emitted in two halves so each mm2
                # pair lands well after the h relu it consumes: 4
                # block-diagonal mm2 column tiles (2 neighbors per strip,
                # K=128), then the shifted relu max(z_nb, -ys) on the DVE
                # (relu(z_nb+ys) = max(z_nb,-ys) + ys; the linear +ys term
                # is folded into the host-side output assembly through the
                # final contraction).
                (t, hs, ysr_sb, z_sbs, ns, st) = f
                if part == 0:
                    st["zp"] = ppool.tile([128, SB], dt.float32, tag="zp",
                                          bufs=2, name="zp")
                zp = st["zp"]
                for c in (0, 1) if part == 0 else (2, 3):
                    nc.tensor.matmul(zp[32 * c: 32 * (c + 1), :ns],
                                     w2a_sb,
                                     hs[c // 2][:, (c % 2) * SB: (c % 2) * SB + ns],
                                     start=True, stop=True,
                                     skip_group_check=True,
                                     tile_position=(0, 32 * c))
                if part == 1:
                    z_sb = zpool.tile([128, SB], dt.bfloat16, tag="z")
                    nc.vector.tensor_tensor(z_sb[:, :ns], zp[:, :ns],
                                            ysr_sb[:, :ns], mybir.AluOpType.max)
                    z_sbs.append(z_sb)

            def emit_out(w):
                # batched final contraction: 4 concurrent single-shot
                # column tiles, then per-strip DMA straight out of PSUM.
                z_sbs, n0, ns = w
                wa_psum = ppool.tile([128, SB], dt.float32, tag="wa", bufs=2,
                                     name="wa_psum")
                for t in range(4):
                    nc.tensor.matmul(wa_psum[32 * t: 32 * (t + 1), :ns],
                                     wg3_sb, z_sbs[t][:, :ns],
                                     start=True, stop=True,
                                     skip_group_check=True,
                                     tile_position=(0, 32 * t))
                if WA_PSUM_DMA:
                    for t in range(4):
                        nc.sync.dma_start(
                            out=out[32 * n0 + 8 * t * ns:
                                    32 * n0 + 8 * (t + 1) * ns].rearrange(
                                "(r n) -> r n", r=8),
                            in_=wa_psum[32 * t: 32 * t + 8, :ns],
                        )
                else:
                    wa_sb = wpool.tile([128, SB], dt.float32, tag="was")
                    nc.vector.tensor_copy(out=wa_sb[:, :ns],
                                          in_=wa_psum[:, :ns])
                    for t in range(4):
                        nc.sync.dma_start(
                            out=out[32 * n0 + 8 * t * ns:
                                    32 * n0 + 8 * (t + 1) * ns].rearrange(
                                "(r n) -> r n", r=8),
                            in_=wa_sb[32 * t: 32 * t + 8, :ns],
                        )

            for n0, ns in SBS:
                # inputs for this superblock (pre-transposed on host); the
                # two ~1MB x chunks go to the two HWDGE rings.
                xt_flat = xt[128 * N * n0: 128 * N * (n0 + ns)].rearrange(
                    "(f m) -> f m", f=128)
                q = N * ns // 4
                xq = []
                for tq in range(4):
                    xq_sb = xpool.tile([128, N * SB // 4], dt.float8e3,
                                       tag="xq", bufs=8)
                    eng = nc.sync if (xrr[0] % 2 == 0) else nc.scalar
                    xrr[0] += 1
                    eng.dma_start(out=xq_sb[:, :q],
                                  in_=xt_flat[:, tq * q: (tq + 1) * q])
                    xq.append(xq_sb)
                ysr_sb = xpool.tile([128, SB], dt.bfloat16, tag="ysr")
                nc.scalar.dma_start(
                    out=ysr_sb[:, :ns],
                    in_=ysr[128 * n0: 128 * (n0 + ns)].rearrange(
                        "(p n) -> p n", p=128),
                )

                if first:
                    # HAM warm-up: ~3.5us of dense matmul right after the
                    # first DMA lands, so the PE clock-gate opens to 2.4GHz
                    # before the real stream starts.
                    first = False
                    warm = ppool.tile([128, 2 * SB], dt.float32, tag="hp",
                                      bufs=2)
                    for _ in range(8):
                        nc.tensor.matmul(warm[0:H1, :ns], wnb_sb,
                                         xq[0][:, :ns], start=True, stop=True)

                z_sbs = []

                # 4 bursts of 4 neighbor-pairs writing 2 double-bank PSUM
                # tiles each; the z-flush of burst t is emitted during
                # burst t+1 (carried across superblocks) so the PE never
                # waits on the relu engines.
                for t in range(4):
                    hs = []
                    for dd in range(2):
                        # interleave the previous burst's z-flush halves
                        # between and after the two hp doubles so each
                        # relu gets a head start before its consumers
                        if dd == 1 and pend is not None:
                            emit_flush(pend, 0)
                        hp = ppool.tile([128, 2 * SB], dt.float32, tag="hp",
                                        bufs=2)
                        for p in range(2):
                            for c in range(2):
                                jj = 4 * dd + 2 * p + c
                                nc.tensor.matmul(
                                    hp[H1 * c: H1 * (c + 1),
                                       p * SB: p * SB + ns],
                                    wnb_sb,
                                    xq[t][:, jj * ns: (jj + 1) * ns],
                                    start=True, stop=True,
                                    tile_position=(0, H1 * c),
                                )
                        h_sb = hpool.tile([128, 2 * SB], dt.bfloat16, tag="h")
                        r = relu_rr[0] = (relu_rr[0] + 1) % 8
                        if r not in (0, 3, 6):
                            nc.scalar.activation(h_sb[:], hp[:],
                                                 RELU, bias=bnb_sb[:],
                                                 scale=1.0)
                        else:
                            nc.vector.tensor_scalar(
                                h_sb[:], hp[:],
                                bnb_sb[:], 0.0,
                                mybir.AluOpType.add, mybir.AluOpType.max)
                        hs.append(h_sb)
                    if pend is not None:
                        emit_flush(pend, 1)
                        pend = None
                    if pend_out is not None and t == 2:
                        emit_out(pend_out)
                        pend_out = None
                    pend = (t, hs, ysr_sb, z_sbs, ns, {})
                pend_out = (z_sbs, n0, ns)

            emit_flush(pend, 0)
            emit_flush(pend, 1)
            emit_out(pend_out)

    nc.compile()
    return nc


def _prep_weights(W_nb, b_nb, W_self, b_self, W_a1, b_a1, W_a2, b_a2):
    """Pack the dense weights into the layouts the kernel expects."""
    W_a1a = W_a1[:H1]          # [64, 16]

    # mm2 block-diagonal: strip rows 0-15 <- h of the even neighbor
    # (moving rows 0-63), rows 16-31 <- odd neighbor (rows 64-127).
    w2a = np.zeros((128, 32), np.float32)
    w2a[:H1, :H2] = W_a1a
    w2a[H1:, H2:] = W_a1a

    # mm3 gather: wa strip row j=2c+d <- a2 . z rows [32c+16d .. +16)
    wg3 = np.zeros((128, 32), np.float32)
    for c in range(4):
        for dd in range(2):
            wg3[32 * c + 16 * dd: 32 * c + 16 * dd + H2, 2 * c + dd] = W_a2[:, 0]

    bnb = np.concatenate([b_nb, b_nb]).reshape(128, 1).astype(np.float32)

    wp = np.zeros((128, 128), np.float32)
    wp[:, 0:H1] = np.asarray(W_nb, np.float32) / XSCALE  # e3m4 prescale undone
    wp[:, 64:96] = w2a
    wp[:, 96:128] = wg3
    return {
        "wpack": wp.astype(BF16),
        "bnb": bnb,
    }


def _prep_core_inputs(x_core, x2_core, W_self, b_self, W_a1b, b_a1, W_a2):
    """Quantize input1 to e3m4 (x2 scale, clipped) in [F, nbr, node]
    layout per superblock; compute the self path ys = z_self + b_a1 on
    host.  The device gets -ys (bf16, replicated to all 8 strip slots)
    for the shifted relu; the linear a2.ys part of the output comes back
    as a per-node host-side addend."""
    xp = np.zeros((B_PAD, N, F), E3M4)
    xp[:B_SH] = np.clip(x_core * XSCALE, -E3MAX, E3MAX).astype(E3M4)
    h_self = np.maximum(x2_core @ W_self + b_self, 0.0)
    ysq = (h_self @ W_a1b + b_a1).astype(BF16)               # [B_SH, H2]
    wext = ysq.astype(np.float32) @ W_a2[:, 0]               # [B_SH]
    negys = np.zeros((B_PAD, H2), BF16)
    negys[:B_SH] = -ysq
    rep = np.tile(negys.T, (8, 1))                           # [128, B_PAD]

    xt_parts = []
    ys_parts = []
    for n0, ns in SBS:
        xt_parts.append(np.ascontiguousarray(
            xp[n0:n0 + ns].transpose(2, 1, 0)).reshape(-1))
        ys_parts.append(np.ascontiguousarray(rep[:, n0:n0 + ns]).reshape(-1))
    return np.concatenate(xt_parts), np.concatenate(ys_parts), wext


def kernel(input1, input2, W_nb, b_nb, W_self, b_self, W_a1, b_a1, W_a2, b_a2):
    global last_results
    if "nc" not in _cache:
        _cache["nc"] = _build_graph()
    nc = _cache["nc"]

    input1 = np.asarray(input1, np.float32)
    input2 = np.asarray(input2, np.float32)
    W_self = np.asarray(W_self, np.float32)
    b_self = np.asarray(b_self, np.float32)
    W_a1 = np.asarray(W_a1, np.float32)
    b_a1 = np.asarray(b_a1, np.float32)
    b_a2 = np.asarray(b_a2, np.float32)
    wmap = _prep_weights(
        np.asarray(W_nb, np.float32), np.asarray(b_nb, np.float32),
        W_self, b_self, W_a1, b_a1,
        np.asarray(W_a2, np.float32), b_a2)

    in_maps = []
    wexts = []
    for c in range(N_CORES):
        xt_c, ys_c, wext_c = _prep_core_inputs(
            input1[c * B_SH: (c + 1) * B_SH],
            input2[c * B_SH: (c + 1) * B_SH],
            W_self, b_self, W_a1[H1:], b_a1,
            np.asarray(W_a2, np.float32))
        m = dict(wmap)
        m["xt"] = xt_c
        m["ysr"] = ys_c
        in_maps.append(m)
        wexts.append(wext_c)

    res = run_bass_kernel_spmd(nc, in_maps, core_ids=list(range(N_CORES)),
                               trace=TRACE)
    last_results = res

    # device emits, per superblock, a [4, 8, ns] = [neighbor, node] block;
    # transpose back to [node, neighbor] row-major and add b_a2 here.
    out = np.empty((B * N, 1), np.float32)
    for c in range(N_CORES):
        oc = res.results[c]["out"]
        core_mat = np.empty((B_SH, N), np.float32)
        for n0, ns in SBS:
            if n0 >= B_SH:
                break
            nv = min(ns, B_SH - n0)
            blk = oc[32 * n0: 32 * n0 + 32 * ns].reshape(N, ns)
            core_mat[n0:n0 + nv] = blk[:, :nv].T
        core_mat += wexts[c][:, None] + b_a2[0]
        out[c * R_SH: (c + 1) * R_SH, 0] = core_mat.reshape(-1)
    return out


# revision 15
# speedup vs baseline: 1.1370x; 1.1370x over previous
"""Trainium2 Bass kernel for GNN message-passing attention MLP.

Computation (per node b with N=32 neighbors, F=128 features):
  h_nb   = relu(input1 @ W_nb + b_nb)          [B,N,H1]
  h_self = relu(input2 @ W_self + b_self)      [B,H1]
  z      = relu(h_nb @ W_a1[:H1] + h_self @ W_a1[H1:] + b_a1)   [B,N,H2]
  out    = (z @ W_a2 + b_a2).reshape(B*N, 1)

Strategy: data-parallel over 8 NeuronCores (6250 nodes each, padded to
6272).  Host-side prep quantizes input1 to fp8-e3m4 (x2 scale, clipped;
the 1/2 folded into W_nb) and pre-transposes to [F, neighbor, node]
layout, halving the dominant HBM stream; the tiny self path
ys = h_self @ W_a1[H1:] + b_a1 is computed on host.  On device, nodes
ride the matmul free dim (512-wide superblocks, one 8-neighbor x tile
per burst alternating across the two HWDGE rings).  mm1 runs as
column-tiled fp8 pairs into double-bank PSUM tiles (one relu
instruction covers 4 neighbors); z tiles pack 8 neighbors (2 per
32-partition strip via block-diagonal stationaries, K=128) and are
evacuated with the shifted relu max(z_nb, -ys) on the DVE — using
relu(z_nb + ys) = max(z_nb, -ys) + ys, whose linear +ys term commutes
with the final H2-contraction and is added per-node on the host
together with b_a2.  The final contraction runs as one batched group
of 4 concurrent column tiles; its [neighbor, node] strips are
re-transposed on the host.  The per-burst z-flush is emitted in two
halves interleaved into the next burst so the PE never stalls on the
relu engines.
"""

import sys
import types

import numpy as np
import ml_dtypes

import concourse.bass as bass
import concourse.mybir as mybir
from concourse import bacc
from concourse.tile import TileContext
from concourse.bass_utils import run_bass_kernel_spmd


def _ensure_axon_hooks():
    """bass_utils' trace path imports antenv.axon_hooks, which this image
    lacks; synthesize it (wired to the PJRT plugin's NTFF profiler) so a
    BASS_TRACE=1 environment doesn't crash the run."""
    try:
        import antenv.axon_hooks  # noqa: F401
        return
    except ImportError:
        pass
    try:
        import antenv
        mod = types.ModuleType("antenv.axon_hooks")
        holder = [None]
        mod.set_axon_ntff_profile_hook = lambda h: holder.__setitem__(0, h)
        mod.get_axon_ntff_profile_hook = lambda: holder[0]
        sys.modules["antenv.axon_hooks"] = mod
        antenv.axon_hooks = mod
        try:
            from trn_agent_boot.trn_boot import _ntff_profile_via_ctypes
            mod.set_axon_ntff_profile_hook(
                _ntff_profile_via_ctypes("/opt/axon/libaxon_pjrt.so"))
        except Exception:
            pass
    except Exception:
        pass


_ensure_axon_hooks()

BF16 = ml_dtypes.bfloat16
E3M4 = ml_dtypes.float8_e3m4
XSCALE = 2.0                   # input1 prescale before e3m4 cast
E3MAX = 15.5                   # e3m4 saturation
WA_PSUM_DMA = False            # PSUM-source DMA rejected by bass; use DVE copy

B, N, F = 50000, 32, 128
H1, H2 = 64, 16
N_CORES = 8
B_SH = B // N_CORES            # 6250 nodes per core
B_PAD = 6272                   # padded to 49*128
SB = 512                       # superblock: nodes per compute block
SBS = [(s * SB, SB) for s in range(B_PAD // SB)]
_rem = B_PAD - (B_PAD // SB) * SB
if _rem:
    SBS.append(((B_PAD // SB) * SB, _rem))
R_PAD = B_PAD * N              # padded rows per core (200704)
R_SH = B_SH * N                # valid rows per core (200000)

_cache = {}
last_results = None  # BassKernelResults of the most recent run (for test harness)
TRACE = False        # set True from test harness to capture an HW profile


def _build_graph():
    dt = mybir.dt
    nc = bacc.Bacc("TRN2", target_bir_lowering=False, debug=False,
                   num_devices=N_CORES)

    xt = nc.declare_dram_parameter("xt", [128 * N * B_PAD], dt.float8e3, isOutput=False)
    ysr = nc.declare_dram_parameter("ysr", [128 * B_PAD], dt.bfloat16, isOutput=False)
    wpack = nc.declare_dram_parameter("wpack", [128, 128], dt.bfloat16, isOutput=False)
    bnb = nc.declare_dram_parameter("bnb", [128, 1], dt.float32, isOutput=False)
    out = nc.declare_dram_parameter("out", [R_PAD], dt.float32, isOutput=True)

    RELU = mybir.ActivationFunctionType.Relu

    with TileContext(nc) as tc:
        with tc.tile_pool(name="const", bufs=1) as cpool, \
             tc.tile_pool(name="xp", bufs=3) as xpool, \
             tc.tile_pool(name="hp", bufs=8) as hpool, \
             tc.tile_pool(name="zs", bufs=10) as zpool, \
             tc.tile_pool(name="wp", bufs=2) as wpool, \
             tc.tile_pool(name="psum", bufs=1, space="PSUM") as ppool:
            # PSUM budget (8 banks): hp double x2 (4), zp x2, wa x2.

            wpack_sb = cpool.tile([128, 128], dt.bfloat16)
            nc.scalar.dma_start(out=wpack_sb[:], in_=wpack[:])
            wnb_sb = wpack_sb[:, 0:H1]
            w2a_sb = wpack_sb[:, 64:96]
            wg3_sb = wpack_sb[:, 96:128]
            bnb_sb = cpool.tile([128, 1], dt.float32)
            nc.scalar.dma_start(out=bnb_sb[:], in_=bnb[:])

            first = True
            relu_rr = [0]     # round-robin for h relu engine assignment
            xrr = [0]         # round-robin for x chunk DMA ring assignment
            pend = None       # deferred z-flush of the previous burst
            pend_out = None   # deferred mm3 + wa evac of the previous sb

            def emit_flush(f, part):
                # z-tile for one burst, emitted in two halves so each mm2
                # pair lands well after the h relu it consumes: 4
                # block-diagonal mm2 column tiles (2 neighbors per strip,
                # K=128), then the shifted relu max(z_nb, -ys) on the DVE
                # (relu(z_nb+ys) = max(z_nb,-ys) + ys; the linear +ys term
                # is folded into the host-side output assembly through the
                # final contraction).
                (t, hs, ysr_sb, z_sbs, ns, st) = f
                if part == 0:
                    st["zp"] = ppool.tile([128, SB], dt.float32, tag="zp",
                                          bufs=2, name="zp")
                zp = st["zp"]
                for c in (0, 1) if part == 0 else (2, 3):
                    nc.tensor.matmul(zp[32 * c: 32 * (c + 1), :ns],
                                     w2a_sb,
                                     hs[c // 2][:, (c % 2) * SB: (c % 2) * SB + ns],
                                     start=True, stop=True,
                                     skip_group_check=True,
                                     tile_position=(0, 32 * c))
                if part == 1:
                    z_sb = zpool.tile([128, SB], dt.bfloat16, tag="z")
                    nc.vector.tensor_tensor(z_sb[:, :ns], zp[:, :ns],
                                            ysr_sb[:, :ns], mybir.AluOpType.max)
                    z_sbs.append(z_sb)

            def emit_out(w):
                # batched final contraction: 4 concurrent single-shot
                # column tiles, then per-strip DMA straight out of PSUM.
                z_sbs, n0, ns = w
                wa_psum = ppool.tile([128, SB], dt.float32, tag="wa", bufs=2,
                                     name="wa_psum")
                for t in range(4):
                    nc.tensor.matmul(wa_psum[32 * t: 32 * (t + 1), :ns],
                                     wg3_sb, z_sbs[t][:, :ns],
                                     start=True, stop=True,
                                     skip_group_check=True,
                                     tile_position=(0, 32 * t))
                if WA_PSUM_DMA:
                    for t in range(4):
                        nc.sync.dma_start(
                            out=out[32 * n0 + 8 * t * ns:
                                    32 * n0 + 8 * (t + 1) * ns].rearrange(
                                "(r n) -> r n", r=8),
                            in_=wa_psum[32 * t: 32 * t + 8, :ns],
                        )
                else:
                    wa_sb = wpool.tile([128, SB], dt.float32, tag="was")
                    nc.vector.tensor_copy(out=wa_sb[:, :ns],
                                          in_=wa_psum[:, :ns])
                    for t in range(4):
                        nc.sync.dma_start(
                            out=out[32 * n0 + 8 * t * ns:
                                    32 * n0 + 8 * (t + 1) * ns].rearrange(
                                "(r n) -> r n", r=8),
                            in_=wa_sb[32 * t: 32 * t + 8, :ns],
                        )

            for n0, ns in SBS:
                # inputs for this superblock (pre-transposed on host); the
                # two ~1MB x chunks go to the two HWDGE rings.
                xt_flat = xt[128 * N * n0: 128 * N * (n0 + ns)].rearrange(
                    "(f m) -> f m", f=128)
                q = N * ns // 4
                xq = []
                for tq in range(4):
                    xq_sb = xpool.tile([128, N * SB // 4], dt.float8e3,
                                       tag="xq", bufs=8)
                    eng = nc.sync if (xrr[0] % 2 == 0) else nc.scalar
                    xrr[0] += 1
                    eng.dma_start(out=xq_sb[:, :q],
                                  in_=xt_flat[:, tq * q: (tq + 1) * q])
                    xq.append(xq_sb)
                ysr_sb = xpool.tile([128, SB], dt.bfloat16, tag="ysr")
                nc.scalar.dma_start(
                    out=ysr_sb[:, :ns],
                    in_=ysr[128 * n0: 128 * (n0 + ns)].rearrange(
                        "(p n) -> p n", p=128),
                )

                if first:
                    # HAM warm-up: ~3.5us of dense matmul right after the
                    # first DMA lands, so the PE clock-gate opens to 2.4GHz
                    # before the real stream starts.
                    first = False
                    warm = ppool.tile([128, 2 * SB], dt.float32, tag="hp",
                                      bufs=2)
                    for _ in range(8):
                        nc.tensor.matmul(warm[0:H1, :ns], wnb_sb,
                                         xq[0][:, :ns], start=True, stop=True)

                z_sbs = []

                # 4 bursts of 4 neighbor-pairs writing 2 double-bank PSUM
                # tiles each; the z-flush of burst t is emitted during
                # burst t+1 (carried across superblocks) so the PE never
                # waits on the relu engines.
                for t in range(4):
                    hs = []
                    for dd in range(2):
                        # interleave the previous burst's z-flush halves
                        # between and after the two hp doubles so each
                        # relu gets a head start before its consumers
                        if dd == 1 and pend is not None:
                            emit_flush(pend, 0)
                        hp = ppool.tile([128, 2 * SB], dt.float32, tag="hp",
                                        bufs=2)
                        # c-major order: consecutive matmuls at the same
                        # tile position share an identical stationary AP so
                        # walrus elides the repeated LDWEIGHTS
                        for c in range(2):
                            for p in range(2):
                                jj = 4 * dd + 2 * p + c
                                nc.tensor.matmul(
                                    hp[H1 * c: H1 * (c + 1),
                                       p * SB: p * SB + ns],
                                    wnb_sb,
                                    xq[t][:, jj * ns: (jj + 1) * ns],
                                    start=True, stop=True,
                                    tile_position=(0, H1 * c),
                                )
                        h_sb = hpool.tile([128, 2 * SB], dt.bfloat16, tag="h")
                        r = relu_rr[0] = (relu_rr[0] + 1) % 8
                        if r not in (0, 3, 6):
                            nc.scalar.activation(h_sb[:], hp[:],
                                                 RELU, bias=bnb_sb[:],
                                                 scale=1.0)
                        else:
                            nc.vector.tensor_scalar(
                                h_sb[:], hp[:],
                                bnb_sb[:], 0.0,
                                mybir.AluOpType.add, mybir.AluOpType.max)
                        hs.append(h_sb)
                    if pend is not None:
                        emit_flush(pend, 1)
                        pend = None
                    if pend_out is not None and t == 2:
                        emit_out(pend_out)
                        pend_out = None
                    pend = (t, hs, ysr_sb, z_sbs, ns, {})
                pend_out = (z_sbs, n0, ns)

            emit_flush(pend, 0)
            emit_flush(pend, 1)
            emit_out(pend_out)

    nc.compile()
    return nc


def _prep_weights(W_nb, b_nb, W_self, b_self, W_a1, b_a1, W_a2, b_a2):
    """Pack the dense weights into the layouts the kernel expects."""
    W_a1a = W_a1[:H1]          # [64, 16]

    # mm2 block-diagonal: strip rows 0-15 <- h of the even neighbor
    # (moving rows 0-63), rows 16-31 <- odd neighbor (rows 64-127).
    w2a = np.zeros((128, 32), np.float32)
    w2a[:H1, :H2] = W_a1a
    w2a[H1:, H2:] = W_a1a

    # mm3 gather: wa strip row j=2c+d <- a2 . z rows [32c+16d .. +16)
    wg3 = np.zeros((128, 32), np.float32)
    for c in range(4):
        for dd in range(2):
            wg3[32 * c + 16 * dd: 32 * c + 16 * dd + H2, 2 * c + dd] = W_a2[:, 0]

    bnb = np.concatenate([b_nb, b_nb]).reshape(128, 1).astype(np.float32)

    wp = np.zeros((128, 128), np.float32)
    wp[:, 0:H1] = np.asarray(W_nb, np.float32) / XSCALE  # e3m4 prescale undone
    wp[:, 64:96] = w2a
    wp[:, 96:128] = wg3
    return {
        "wpack": wp.astype(BF16),
        "bnb": bnb,
    }


def _prep_core_inputs(x_core, x2_core, W_self, b_self, W_a1b, b_a1, W_a2):
    """Quantize input1 to e3m4 (x2 scale, clipped) in [F, nbr, node]
    layout per superblock; compute the self path ys = z_self + b_a1 on
    host.  The device gets -ys (bf16, replicated to all 8 strip slots)
    for the shifted relu; the linear a2.ys part of the output comes back
    as a per-node host-side addend."""
    xp = np.zeros((B_PAD, N, F), E3M4)
    xp[:B_SH] = np.clip(x_core * XSCALE, -E3MAX, E3MAX).astype(E3M4)
    h_self = np.maximum(x2_core @ W_self + b_self, 0.0)
    ysq = (h_self @ W_a1b + b_a1).astype(BF16)               # [B_SH, H2]
    wext = ysq.astype(np.float32) @ W_a2[:, 0]               # [B_SH]
    negys = np.zeros((B_PAD, H2), BF16)
    negys[:B_SH] = -ysq
    rep = np.tile(negys.T, (8, 1))                           # [128, B_PAD]

    xt_parts = []
    ys_parts = []
    for n0, ns in SBS:
        xt_parts.append(np.ascontiguousarray(
            xp[n0:n0 + ns].transpose(2, 1, 0)).reshape(-1))
        ys_parts.append(np.ascontiguousarray(rep[:, n0:n0 + ns]).reshape(-1))
    return np.concatenate(xt_parts), np.concatenate(ys_parts), wext


def kernel(input1, input2, W_nb, b_nb, W_self, b_self, W_a1, b_a1, W_a2, b_a2):
    global last_results
    if "nc" not in _cache:
        _cache["nc"] = _build_graph()
    nc = _cache["nc"]

    input1 = np.asarray(input1, np.float32)
    input2 = np.asarray(input2, np.float32)
    W_self = np.asarray(W_self, np.float32)
    b_self = np.asarray(b_self, np.float32)
    W_a1 = np.asarray(W_a1, np.float32)
    b_a1 = np.asarray(b_a1, np.float32)
    b_a2 = np.asarray(b_a2, np.float32)
    wmap = _prep_weights(
        np.asarray(W_nb, np.float32), np.asarray(b_nb, np.float32),
        W_self, b_self, W_a1, b_a1,
        np.asarray(W_a2, np.float32), b_a2)

    in_maps = []
    wexts = []
    for c in range(N_CORES):
        xt_c, ys_c, wext_c = _prep_core_inputs(
            input1[c * B_SH: (c + 1) * B_SH],
            input2[c * B_SH: (c + 1) * B_SH],
            W_self, b_self, W_a1[H1:], b_a1,
            np.asarray(W_a2, np.float32))
        m = dict(wmap)
        m["xt"] = xt_c
        m["ysr"] = ys_c
        in_maps.append(m)
        wexts.append(wext_c)

    res = run_bass_kernel_spmd(nc, in_maps, core_ids=list(range(N_CORES)),
                               trace=TRACE)
    last_results = res

    # device emits, per superblock, a [4, 8, ns] = [neighbor, node] block;
    # transpose back to [node, neighbor] row-major and add b_a2 here.
    out = np.empty((B * N, 1), np.float32)
    for c in range(N_CORES):
        oc = res.results[c]["out"]
        core_mat = np.empty((B_SH, N), np.float32)
        for n0, ns in SBS:
            if n0 >= B_SH:
                break
            nv = min(ns, B_SH - n0)
            blk = oc[32 * n0: 32 * n0 + 32 * ns].reshape(N, ns)
            core_mat[n0:n0 + nv] = blk[:, :nv].T
        core_mat += wexts[c][:, None] + b_a2[0]
        out[c * R_SH: (c + 1) * R_SH, 0] = core_mat.reshape(-1)
    return out
